# revision 5
# baseline (speedup 1.0000x reference)
# Trainium2 Bass kernel for nn_CFTAuxHead (bilinear 4x resize + bbox
# rasterization + MSE loss), data-parallel over batch across 8 NeuronCores.
#
# Math summary (per sample):
#   feat_up = A^T @ feat @ A  (A = exact 160->640 bilinear weight matrix)
#   heatmap = last-writer-wins paint of 128 axis-aligned rects (value z_n)
#   loss    = mean((feat_up - heatmap)^2) over all pixels
#
# Rasterization on device: 5 "paint" matmuls over box interval-indicator
# matrices U[n, row], V[n, col] with per-box weights:
#   S_lo = sum_n 2^(n')   (n' = n mod 64, boxes n < 64)      [exponent encode]
#   S_hi = sum_n 2^(n')   (boxes n >= 64)
#   A_lo/A_hi = same with z_n * 2^(n')
#   M0   = sum_n z_n
# Per-pixel decode (exact when coverage depth <= 2, clamped otherwise):
#   C  = S_hi + 2^-64 * S_lo        CA = A_hi + 2^-64 * A_lo
#   E  = C & 0xFF800000             (isolates 2^(top box index), exact)
#   Z  = clamp((CA - (C - E) * M0) / (2E - C), -2, 2);  Z = 0 if uncovered
# loss contribution = (feat_up - Z)^2, reduced on-chip to one scalar per core.

import os
import numpy as np

B, C_IN, H, W = 32, 1, 160, 160
UP = 4
HO, WO = H * UP, W * UP
NBOX = 128
NCORES = 8
SPC = B // NCORES  # samples per core
NPIX = float(B * HO * WO)

_CACHE = {}


def _resize_matrix():
    """Exact bilinear (half-pixel centers, edge-clamped) 160->640 matrix,
    matching jax.image.resize(method='bilinear') for upsampling."""
    n_in, n_out = H, HO
    scale = n_out / n_in
    x = (np.arange(n_out, dtype=np.float64) + 0.5) / scale - 0.5
    k = np.arange(n_in, dtype=np.float64)
    w = np.maximum(0.0, 1.0 - np.abs(x[None, :] - k[:, None]))  # [in, out]
    w = w / w.sum(axis=0, keepdims=True)
    return w.astype(np.float32)


def _build(krep=1):
    import concourse.bacc as bacc
    import concourse.mybir as mybir
    from concourse.tile import TileContext

    fp32 = mybir.dt.float32
    i32 = mybir.dt.int32
    Alu = mybir.AluOpType

    nc = bacc.Bacc("TRN2", target_bir_lowering=False, debug=False,
                   enable_asserts=False, num_devices=NCORES)
    feat_d = nc.dram_tensor("feat", [SPC, H, W], fp32, kind="ExternalInput")
    box_d = nc.dram_tensor("boxes", [SPC, NBOX, 5], fp32, kind="ExternalInput")
    amat_d = nc.dram_tensor("amat", [H, HO], fp32, kind="ExternalInput")
    out_d = nc.dram_tensor("out", [1, 1], fp32, kind="ExternalOutput")

    TAIL = float(2.0 ** -64)
    MASK_EXP = -8388608  # 0xFF800000 as signed int32

    with TileContext(nc, num_cores=NCORES) as tc:
        with tc.tile_pool(name="const", bufs=1) as cpool, \
             tc.tile_pool(name="samp", bufs=2) as spool, \
             tc.tile_pool(name="dec", bufs=3) as dpool, \
             tc.tile_pool(name="ps", bufs=1, space="PSUM") as ppool, \
             tc.tile_pool(name="psf", bufs=1, space="PSUM") as fpool:

            # ---- constants ----
            A0 = cpool.tile([128, HO], fp32, tag="A0")
            A1 = cpool.tile([32, HO], fp32, tag="A1")
            nc.sync.dma_start(A0[:], amat_d.ap()[0:128, :])
            nc.sync.dma_start(A1[:], amat_d.ap()[128:160, :])

            iota_i = cpool.tile([128, HO], i32, tag="ioti")
            nc.gpsimd.iota(iota_i[:], pattern=[[1, HO]], base=0,
                           channel_multiplier=0)
            iota_f = cpool.tile([128, HO], fp32, tag="iotf")
            nc.vector.tensor_copy(iota_f[:], iota_i[:])

            nidx_i = cpool.tile([128, 1], i32, tag="nidxi")
            nc.gpsimd.iota(nidx_i[:], pattern=[[1, 1]], base=1,
                           channel_multiplier=1)  # n' = n+1 in 1..128
            nidx_f = cpool.tile([128, 1], fp32, tag="nidxf")
            nc.vector.tensor_copy(nidx_f[:], nidx_i[:])

            ones_t = cpool.tile([128, 1], fp32, tag="ones")
            nc.vector.memset(ones_t[:], 1.0)

            # group masks and exponent weights
            glo = cpool.tile([128, 1], fp32, tag="glo")
            nc.vector.tensor_scalar(glo[:], nidx_f[:], 64.0, None, Alu.is_le)
            ghi = cpool.tile([128, 1], fp32, tag="ghi")
            nc.vector.tensor_scalar(ghi[:], nidx_f[:], 64.0, None, Alu.is_gt)

            wslo_b = cpool.tile([128, 1], i32, tag="wslob")
            nc.vector.tensor_scalar(wslo_b[:], nidx_i[:], 126, None, Alu.add)
            nc.vector.tensor_scalar(wslo_b[:], wslo_b[:], 23, None,
                                    Alu.logical_shift_left)
            wshi_b = cpool.tile([128, 1], i32, tag="wshib")
            nc.vector.tensor_scalar(wshi_b[:], nidx_i[:], 62, None, Alu.add)
            nc.vector.tensor_scalar(wshi_b[:], wshi_b[:], 23, None,
                                    Alu.logical_shift_left)
            wslo = cpool.tile([128, 1], fp32, tag="wslo")
            nc.vector.tensor_tensor(wslo[:], wslo_b[:].bitcast(fp32), glo[:],
                                    Alu.mult)
            wshi = cpool.tile([128, 1], fp32, tag="wshi")
            nc.vector.tensor_tensor(wshi[:], wshi_b[:].bitcast(fp32), ghi[:],
                                    Alu.mult)

            accbuf = cpool.tile([128, krep * SPC * 10], fp32, tag="acc")

            def floor_pos(src_ap, tagp):
                """floor(x) for 0 <= x < 2^23, robust to convert rounding."""
                ti = dpool.tile([128, 1], i32, tag=tagp + "_i")
                nc.vector.tensor_copy(ti[:], src_ap)
                tf = dpool.tile([128, 1], fp32, tag=tagp + "_f")
                nc.vector.tensor_copy(tf[:], ti[:])
                m = dpool.tile([128, 1], fp32, tag=tagp + "_m")
                nc.vector.tensor_tensor(m[:], tf[:], src_ap, Alu.is_gt)
                fl = dpool.tile([128, 1], fp32, tag=tagp + "_o")
                nc.vector.tensor_tensor(fl[:], tf[:], m[:], Alu.subtract)
                return fl

            for rep in range(krep):
                for s in range(SPC):
                    # ---- load feat, resize step 1: out1 = F^T A ----
                    F0 = spool.tile([128, W], fp32, tag="F0")
                    F1 = spool.tile([32, W], fp32, tag="F1")
                    nc.sync.dma_start(F0[:], feat_d.ap()[s, 0:128, :])
                    nc.sync.dma_start(F1[:], feat_d.ap()[s, 128:160, :])

                    out1a = spool.tile([128, HO], fp32, tag="out1a")
                    out1b = spool.tile([32, HO], fp32, tag="out1b")
                    for mc, (msz, o1) in enumerate([(128, out1a), (32, out1b)]):
                        moff = 0 if mc == 0 else 128
                        for hh in range(2):
                            hs = slice(hh * 320, (hh + 1) * 320)
                            p1 = fpool.tile([128, 320], fp32, tag="paux")
                            nc.tensor.matmul(
                                p1[0:msz, :], F0[:, moff:moff + msz], A0[:, hs],
                                start=True, stop=False)
                            nc.tensor.matmul(
                                p1[0:msz, :], F1[:, moff:moff + msz], A1[:, hs],
                                start=False, stop=True)
                            nc.scalar.copy(o1[:, hs], p1[0:msz, :])

                    # ---- box prep ----
                    bx = spool.tile([128, 5], fp32, tag="bx")
                    nc.sync.dma_start(bx[:], box_d.ap()[s])
                    xq = bx[:, 0:1]
                    yq = bx[:, 1:2]
                    zq = bx[:, 2:3]
                    wq = bx[:, 3:4]
                    lq = bx[:, 4:5]

                    w2 = dpool.tile([128, 1], fp32, tag="w2")
                    nc.vector.tensor_scalar(w2[:], wq, 0.5, None, Alu.mult)
                    l2 = dpool.tile([128, 1], fp32, tag="l2")
                    nc.vector.tensor_scalar(l2[:], lq, 0.5, None, Alu.mult)

                    cx = floor_pos(xq, "cx")
                    cy = floor_pos(yq, "cy")
                    hw = floor_pos(w2[:], "hw")
                    hl = floor_pos(l2[:], "hl")
                    nc.vector.tensor_scalar(hw[:], hw[:], 3.0, None, Alu.max)
                    nc.vector.tensor_scalar(hl[:], hl[:], 3.0, None, Alu.max)

                    xmin = dpool.tile([128, 1], fp32, tag="xmin")
                    nc.vector.tensor_tensor(xmin[:], cx[:], hw[:], Alu.subtract)
                    nc.vector.tensor_scalar(xmin[:], xmin[:], 0.0, None, Alu.max)
                    xmax = dpool.tile([128, 1], fp32, tag="xmax")
                    nc.vector.tensor_tensor(xmax[:], cx[:], hw[:], Alu.add)
                    nc.vector.tensor_scalar(xmax[:], xmax[:], 1.0, float(HO),
                                            Alu.add, Alu.min)
                    ymin = dpool.tile([128, 1], fp32, tag="ymin")
                    nc.vector.tensor_tensor(ymin[:], cy[:], hl[:], Alu.subtract)
                    nc.vector.tensor_scalar(ymin[:], ymin[:], 0.0, None, Alu.max)
                    ymax = dpool.tile([128, 1], fp32, tag="ymax")
                    nc.vector.tensor_tensor(ymax[:], cy[:], hl[:], Alu.add)
                    nc.vector.tensor_scalar(ymax[:], ymax[:], 1.0, float(WO),
                                            Alu.add, Alu.min)

                    # validity (w > 0 and l > 0) folded into U weights
                    vw = dpool.tile([128, 1], fp32, tag="vw")
                    nc.vector.tensor_scalar(vw[:], wq, 0.0, None, Alu.is_gt)
                    vl = dpool.tile([128, 1], fp32, tag="vl")
                    nc.vector.tensor_scalar(vl[:], lq, 0.0, None, Alu.is_gt)
                    vv = dpool.tile([128, 1], fp32, tag="vv")
                    nc.vector.tensor_tensor(vv[:], vw[:], vl[:], Alu.mult)

                    # per-box paint weights (valid-masked)
                    wslo_v = dpool.tile([128, 1], fp32, tag="wslov")
                    nc.vector.tensor_tensor(wslo_v[:], wslo[:], vv[:], Alu.mult)
                    wshi_v = dpool.tile([128, 1], fp32, tag="wshiv")
                    nc.vector.tensor_tensor(wshi_v[:], wshi[:], vv[:], Alu.mult)
                    walo = dpool.tile([128, 1], fp32, tag="walo")
                    nc.vector.tensor_tensor(walo[:], wslo_v[:], zq, Alu.mult)
                    wahi = dpool.tile([128, 1], fp32, tag="wahi")
                    nc.vector.tensor_tensor(wahi[:], wshi_v[:], zq, Alu.mult)
                    wm0 = dpool.tile([128, 1], fp32, tag="wm0")
                    nc.vector.tensor_tensor(wm0[:], vv[:], zq, Alu.mult)

                    # ---- U (rows) and V (cols) indicators ----
                    tU = spool.tile([128, HO], fp32, tag="tU")
                    nc.vector.tensor_scalar(tU[:], iota_f[:], xmax[:], None,
                                            Alu.is_lt)
                    U = spool.tile([128, HO], fp32, tag="U")
                    nc.vector.scalar_tensor_tensor(
                        U[:], iota_f[:], xmin[:], tU[:],
                        Alu.is_ge, Alu.logical_and)
                    tV = spool.tile([128, HO], fp32, tag="tV")
                    nc.vector.tensor_scalar(tV[:], iota_f[:], ymax[:], None,
                                            Alu.is_lt)
                    V = spool.tile([128, HO], fp32, tag="V")
                    nc.vector.scalar_tensor_tensor(
                        V[:], iota_f[:], ymin[:], tV[:],
                        Alu.is_ge, Alu.logical_and)

                    U_slo = spool.tile([128, HO], fp32, tag="Uslo")
                    nc.vector.tensor_scalar(U_slo[:], U[:], wslo_v[:], None,
                                            Alu.mult)
                    U_shi = spool.tile([128, HO], fp32, tag="Ushi")
                    nc.vector.tensor_scalar(U_shi[:], U[:], wshi_v[:], None,
                                            Alu.mult)
                    U_alo = spool.tile([128, HO], fp32, tag="Ualo")
                    nc.vector.tensor_scalar(U_alo[:], U[:], walo[:], None,
                                            Alu.mult)
                    U_ahi = spool.tile([128, HO], fp32, tag="Uahi")
                    nc.vector.tensor_scalar(U_ahi[:], U[:], wahi[:], None,
                                            Alu.mult)
                    U_m0 = spool.tile([128, HO], fp32, tag="Um0")
                    nc.vector.tensor_scalar(U_m0[:], U[:], wm0[:], None,
                                            Alu.mult)

                    # ---- per-chunk paints + decode + loss ----
                    for m in range(5):
                        ms = slice(m * 128, (m + 1) * 128)
                        for hh in range(2):
                            hs = slice(hh * 320, (hh + 1) * 320)
                            rhsV = V[:, hs]

                            pSlo = ppool.tile([128, 320], fp32, tag="pSlo")
                            nc.tensor.matmul(pSlo[:], U_slo[:, ms], rhsV,
                                             start=True, stop=True)
                            pShi = ppool.tile([128, 320], fp32, tag="pShi")
                            nc.tensor.matmul(pShi[:], U_shi[:, ms], rhsV,
                                             start=True, stop=True)
                            pAlo = ppool.tile([128, 320], fp32, tag="pAlo")
                            nc.tensor.matmul(pAlo[:], U_alo[:, ms], rhsV,
                                             start=True, stop=True)
                            pAhi = ppool.tile([128, 320], fp32, tag="pAhi")
                            nc.tensor.matmul(pAhi[:], U_ahi[:, ms], rhsV,
                                             start=True, stop=True)
                            pM0 = ppool.tile([128, 320], fp32, tag="pM0")
                            nc.tensor.matmul(pM0[:], U_m0[:, ms], rhsV,
                                             start=True, stop=True)

                            pF = fpool.tile([128, 320], fp32, tag="pF")
                            nc.tensor.matmul(pF[:], out1a[:, ms], A0[:, hs],
                                             start=True, stop=False)
                            nc.tensor.matmul(pF[:], out1b[:, ms], A1[:, hs],
                                             start=False, stop=True)

                            Cs = dpool.tile([128, 320], fp32, tag="Cs")
                            nc.vector.tensor_scalar(Cs[:], pSlo[:], TAIL, None,
                                                    Alu.mult)
                            Cs2 = dpool.tile([128, 320], fp32, tag="Cs2")
                            nc.vector.tensor_tensor(Cs2[:], Cs[:], pShi[:],
                                                    Alu.add)
                            CA = dpool.tile([128, 320], fp32, tag="CAt")
                            nc.vector.tensor_scalar(CA[:], pAlo[:], TAIL, None,
                                                    Alu.mult)
                            CA2 = dpool.tile([128, 320], fp32, tag="CA2")
                            nc.vector.tensor_tensor(CA2[:], CA[:], pAhi[:],
                                                    Alu.add)
                            Et = dpool.tile([128, 320], fp32, tag="Et")
                            nc.vector.tensor_scalar(
                                Et[:].bitcast(i32), Cs2[:].bitcast(i32),
                                MASK_EXP, None, Alu.bitwise_and)
                            at = dpool.tile([128, 320], fp32, tag="at")
                            nc.vector.tensor_tensor(at[:], Cs2[:], Et[:],
                                                    Alu.subtract)
                            bt = dpool.tile([128, 320], fp32, tag="bt")
                            nc.vector.tensor_tensor(bt[:], at[:], pM0[:],
                                                    Alu.mult)
                            numer = dpool.tile([128, 320], fp32, tag="numer")
                            nc.vector.tensor_tensor(numer[:], CA2[:], bt[:],
                                                    Alu.subtract)
                            den = dpool.tile([128, 320], fp32, tag="den")
                            nc.vector.scalar_tensor_tensor(
                                den[:], Et[:], 2.0, Cs2[:],
                                Alu.mult, Alu.subtract)
                            nc.vector.tensor_scalar(den[:], den[:], 1e-30,
                                                    None, Alu.max)
                            rden = dpool.tile([128, 320], fp32, tag="rden")
                            nc.vector.reciprocal(rden[:], den[:])
                            Z0 = dpool.tile([128, 320], fp32, tag="Z0")
                            nc.vector.tensor_tensor(Z0[:], numer[:], rden[:],
                                                    Alu.mult)
                            nc.vector.tensor_scalar(Z0[:], Z0[:], -2.0, 2.0,
                                                    Alu.max, Alu.min)
                            dt_ = dpool.tile([128, 320], fp32, tag="dt_")
                            nc.vector.tensor_tensor(dt_[:], pF[:], Z0[:],
                                                    Alu.subtract)
                            dsq = dpool.tile([128, 320], fp32, tag="dsq")
                            idx = ((rep * SPC + s) * 10) + m * 2 + hh
                            nc.vector.scalar_tensor_tensor(
                                dsq[:], dt_[:], 1.0, dt_[:],
                                Alu.bypass, Alu.mult,
                                accum_out=accbuf[:, idx:idx + 1])

            # ---- final reduction ----
            tot = cpool.tile([128, 1], fp32, tag="tot")
            nc.vector.tensor_reduce(
                tot[:], accbuf[:, 0:krep * SPC * 10],
                mybir.AxisListType.X, Alu.add)
            if krep > 1:
                nc.vector.tensor_scalar(tot[:], tot[:], 1.0 / krep, None,
                                        Alu.mult)
            pfin = fpool.tile([128, 320], fp32, tag="paux")
            nc.tensor.matmul(pfin[0:1, 0:1], tot[:], ones_t[:],
                             start=True, stop=True)
            res = cpool.tile([1, 1], fp32, tag="res")
            nc.scalar.copy(res[:], pfin[0:1, 0:1])
            nc.sync.dma_start(out_d.ap(), res[:])

    nc.compile()
    return nc


def _get_nc(krep=1):
    key = ("nc", krep)
    if key not in _CACHE:
        _CACHE[key] = _build(krep)
    return _CACHE[key]


def run_cores(feat, gt_bboxes, krep=1):
    """Run the SPMD kernel; returns list of per-core sum-of-squared-diffs."""
    from concourse.bass_utils import run_bass_kernel_spmd
    nc = _get_nc(krep)
    amat = _resize_matrix()
    feat = np.ascontiguousarray(np.asarray(feat, dtype=np.float32))
    gt = np.ascontiguousarray(np.asarray(gt_bboxes, dtype=np.float32))
    in_maps = []
    for i in range(NCORES):
        sl = slice(i * SPC, (i + 1) * SPC)
        in_maps.append({
            "feat": np.ascontiguousarray(feat[sl, 0]),
            "boxes": np.ascontiguousarray(gt[sl]),
            "amat": amat,
        })
    res = run_bass_kernel_spmd(nc, in_maps, core_ids=list(range(NCORES)))
    return [float(res.results[i]["out"][0, 0]) for i in range(NCORES)]


def kernel(feat, gt_bboxes):
    parts = run_cores(feat, gt_bboxes, krep=1)
    total = float(np.sum(np.asarray(parts, dtype=np.float64)))
    return np.asarray(np.float32(total / NPIX))


# revision 19
# speedup vs baseline: 342.5692x; 342.5692x over previous
# Trainium2 Bass kernel for nn_CFTAuxHead (bilinear 4x resize + bbox
# rasterization + MSE loss), data-parallel over batch across 8 NeuronCores.
#
# Math summary (per sample):
#   feat_up = A^T @ feat @ A  (A = exact 160->640 bilinear weight matrix)
#   heatmap = last-writer-wins paint of 128 axis-aligned rects (value z_n)
#   loss    = mean((feat_up - heatmap)^2) over all pixels
#
# Rasterization on device: 5 "paint" matmuls over box interval-indicator
# matrices U[n, row], V[n, col] with per-box weights:
#   S_lo = sum_n 2^(n')   (n' = n mod 64, boxes n < 64)      [exponent encode]
#   S_hi = sum_n 2^(n')   (boxes n >= 64)
#   A_lo/A_hi = same with z_n * 2^(n')
#   M0   = sum_n z_n
# Per-pixel decode (exact when coverage depth <= 2, clamped otherwise):
#   C  = S_hi + 2^-64 * S_lo        CA = A_hi + 2^-64 * A_lo
#   E  = C & 0xFF800000             (isolates 2^(top box index), exact)
#   Z  = clamp((CA - (C - E) * M0) / (2E - C), -2, 2);  Z = 0 if uncovered
# loss contribution = (feat_up - Z)^2, reduced on-chip to one scalar per core.

import os
import numpy as np

B, C_IN, H, W = 32, 1, 160, 160
UP = 4
HO, WO = H * UP, W * UP
NBOX = 128
NCORES = 8
SPC = B // NCORES  # samples per core
NPIX = float(B * HO * WO)

_CACHE = {}


def _resize_matrix():
    """Exact bilinear (half-pixel centers, edge-clamped) 160->640 matrix,
    matching jax.image.resize(method='bilinear') for upsampling."""
    n_in, n_out = H, HO
    scale = n_out / n_in
    x = (np.arange(n_out, dtype=np.float64) + 0.5) / scale - 0.5
    k = np.arange(n_in, dtype=np.float64)
    w = np.maximum(0.0, 1.0 - np.abs(x[None, :] - k[:, None]))  # [in, out]
    w = w / w.sum(axis=0, keepdims=True)
    return w.astype(np.float32)


def _build(krep=1):
    import concourse.bacc as bacc
    import concourse.mybir as mybir
    from concourse.tile import TileContext

    skip_decode = os.environ.get("KV_SKIP_DECODE", "0") == "1"
    skip_mm = os.environ.get("KV_SKIP_MM", "0") == "1"

    fp32 = mybir.dt.float32
    bf16 = mybir.dt.bfloat16
    i32 = mybir.dt.int32
    Alu = mybir.AluOpType

    nc = bacc.Bacc("TRN2", target_bir_lowering=False, debug=False,
                   enable_asserts=False, num_devices=NCORES)
    feat_d = nc.dram_tensor("feat", [SPC, H, W], fp32, kind="ExternalInput")
    box_d = nc.dram_tensor("boxes", [SPC, NBOX, 5], fp32, kind="ExternalInput")
    amat_d = nc.dram_tensor("amat", [H, HO], fp32, kind="ExternalInput")
    out_d = nc.dram_tensor("out", [1, 1], fp32, kind="ExternalOutput")

    TAIL = float(2.0 ** -64)
    MASK_EXP = -8388608  # 0xFF800000 as signed int32

    with TileContext(nc, num_cores=NCORES) as tc:
        with tc.tile_pool(name="const", bufs=1) as cpool, \
             tc.tile_pool(name="samp", bufs=2) as spool, \
             tc.tile_pool(name="dec", bufs=3) as dpool, \
             tc.tile_pool(name="ps", bufs=1, space="PSUM") as ppool, \
             tc.tile_pool(name="psf", bufs=1, space="PSUM") as fpool:

            # ---- constants ----
            A0 = cpool.tile([128, HO], fp32, tag="A0")
            A1 = cpool.tile([32, HO], fp32, tag="A1")
            nc.sync.dma_start(A0[:], amat_d.ap()[0:128, :])
            nc.sync.dma_start(A1[:], amat_d.ap()[128:160, :])

            iota_i = cpool.tile([128, HO], i32, tag="ioti")
            nc.gpsimd.iota(iota_i[:], pattern=[[1, HO]], base=0,
                           channel_multiplier=0)
            iota_f = cpool.tile([128, HO], fp32, tag="iotf")
            nc.vector.tensor_copy(iota_f[:], iota_i[:])

            nidx_i = cpool.tile([128, 1], i32, tag="nidxi")
            nc.gpsimd.iota(nidx_i[:], pattern=[[1, 1]], base=1,
                           channel_multiplier=1)  # n' = n+1 in 1..128
            nidx_f = cpool.tile([128, 1], fp32, tag="nidxf")
            nc.vector.tensor_copy(nidx_f[:], nidx_i[:])

            ones_t = cpool.tile([128, 1], fp32, tag="ones")
            nc.vector.memset(ones_t[:], 1.0)

            # group masks and exponent weights
            glo = cpool.tile([128, 1], fp32, tag="glo")
            nc.vector.tensor_scalar(glo[:], nidx_f[:], 64.0, None, Alu.is_le)
            ghi = cpool.tile([128, 1], fp32, tag="ghi")
            nc.vector.tensor_scalar(ghi[:], nidx_f[:], 64.0, None, Alu.is_gt)

            wslo_b = cpool.tile([128, 1], i32, tag="wslob")
            nc.vector.tensor_scalar(wslo_b[:], nidx_i[:], 126, None, Alu.add)
            nc.vector.tensor_scalar(wslo_b[:], wslo_b[:], 23, None,
                                    Alu.logical_shift_left)
            wshi_b = cpool.tile([128, 1], i32, tag="wshib")
            nc.vector.tensor_scalar(wshi_b[:], nidx_i[:], 62, None, Alu.add)
            nc.vector.tensor_scalar(wshi_b[:], wshi_b[:], 23, None,
                                    Alu.logical_shift_left)
            wslo = cpool.tile([128, 1], fp32, tag="wslo")
            nc.vector.tensor_tensor(wslo[:], wslo_b[:].bitcast(fp32), glo[:],
                                    Alu.mult)
            wshi = cpool.tile([128, 1], fp32, tag="wshi")
            nc.vector.tensor_tensor(wshi[:], wshi_b[:].bitcast(fp32), ghi[:],
                                    Alu.mult)

            accbuf = cpool.tile([128, krep * SPC * 5], fp32, tag="acc")

            def floor_pos(src_ap, tagp):
                """floor(x) for 0 <= x < 2^23, robust to convert rounding."""
                ti = dpool.tile([128, 1], i32, tag=tagp + "_i")
                nc.vector.tensor_copy(ti[:], src_ap)
                tf = dpool.tile([128, 1], fp32, tag=tagp + "_f")
                nc.vector.tensor_copy(tf[:], ti[:])
                m = dpool.tile([128, 1], fp32, tag=tagp + "_m")
                nc.vector.tensor_tensor(m[:], tf[:], src_ap, Alu.is_gt)
                fl = dpool.tile([128, 1], fp32, tag=tagp + "_o")
                nc.vector.tensor_tensor(fl[:], tf[:], m[:], Alu.subtract)
                return fl

            for rep in range(krep):
                for s in range(SPC):
                    # ---- load feat, resize step 1: out1 = F^T A ----
                    F0 = spool.tile([128, W], fp32, tag="F0")
                    F1 = spool.tile([32, W], fp32, tag="F1")
                    nc.sync.dma_start(F0[:], feat_d.ap()[s, 0:128, :])
                    nc.sync.dma_start(F1[:], feat_d.ap()[s, 128:160, :])

                    out1a = spool.tile([128, HO], fp32, tag="out1a")
                    out1b = spool.tile([32, HO], fp32, tag="out1b")
                    for mc, (msz, o1) in enumerate([(128, out1a), (32, out1b)]):
                        moff = 0 if mc == 0 else 128
                        for hh in range(2):
                            hs = slice(hh * 320, (hh + 1) * 320)
                            p1 = fpool.tile([128, 320], fp32, tag="paux")
                            nc.tensor.matmul(
                                p1[0:msz, :], F0[:, moff:moff + msz], A0[:, hs],
                                start=True, stop=False)
                            nc.tensor.matmul(
                                p1[0:msz, :], F1[:, moff:moff + msz], A1[:, hs],
                                start=False, stop=True)
                            nc.scalar.copy(o1[:, hs], p1[0:msz, :])

                    # ---- box prep ----
                    bx = spool.tile([128, 5], fp32, tag="bx")
                    nc.sync.dma_start(bx[:], box_d.ap()[s])
                    xq = bx[:, 0:1]
                    yq = bx[:, 1:2]
                    zq = bx[:, 2:3]
                    wq = bx[:, 3:4]
                    lq = bx[:, 4:5]

                    w2 = dpool.tile([128, 1], fp32, tag="w2")
                    nc.vector.tensor_scalar(w2[:], wq, 0.5, None, Alu.mult)
                    l2 = dpool.tile([128, 1], fp32, tag="l2")
                    nc.vector.tensor_scalar(l2[:], lq, 0.5, None, Alu.mult)

                    cx = floor_pos(xq, "cx")
                    cy = floor_pos(yq, "cy")
                    hw = floor_pos(w2[:], "hw")
                    hl = floor_pos(l2[:], "hl")
                    nc.vector.tensor_scalar(hw[:], hw[:], 3.0, None, Alu.max)
                    nc.vector.tensor_scalar(hl[:], hl[:], 3.0, None, Alu.max)

                    xmin = dpool.tile([128, 1], fp32, tag="xmin")
                    nc.vector.tensor_tensor(xmin[:], cx[:], hw[:], Alu.subtract)
                    nc.vector.tensor_scalar(xmin[:], xmin[:], 0.0, None, Alu.max)
                    xmax = dpool.tile([128, 1], fp32, tag="xmax")
                    nc.vector.tensor_tensor(xmax[:], cx[:], hw[:], Alu.add)
                    nc.vector.tensor_scalar(xmax[:], xmax[:], 1.0, float(HO),
                                            Alu.add, Alu.min)
                    ymin = dpool.tile([128, 1], fp32, tag="ymin")
                    nc.vector.tensor_tensor(ymin[:], cy[:], hl[:], Alu.subtract)
                    nc.vector.tensor_scalar(ymin[:], ymin[:], 0.0, None, Alu.max)
                    ymax = dpool.tile([128, 1], fp32, tag="ymax")
                    nc.vector.tensor_tensor(ymax[:], cy[:], hl[:], Alu.add)
                    nc.vector.tensor_scalar(ymax[:], ymax[:], 1.0, float(WO),
                                            Alu.add, Alu.min)

                    # validity (w > 0 and l > 0) folded into U weights
                    vw = dpool.tile([128, 1], fp32, tag="vw")
                    nc.vector.tensor_scalar(vw[:], wq, 0.0, None, Alu.is_gt)
                    vl = dpool.tile([128, 1], fp32, tag="vl")
                    nc.vector.tensor_scalar(vl[:], lq, 0.0, None, Alu.is_gt)
                    vv = dpool.tile([128, 1], fp32, tag="vv")
                    nc.vector.tensor_tensor(vv[:], vw[:], vl[:], Alu.mult)

                    # per-box paint weights (valid-masked)
                    wslo_v = dpool.tile([128, 1], fp32, tag="wslov")
                    nc.vector.tensor_tensor(wslo_v[:], wslo[:], vv[:], Alu.mult)
                    wshi_v = dpool.tile([128, 1], fp32, tag="wshiv")
                    nc.vector.tensor_tensor(wshi_v[:], wshi[:], vv[:], Alu.mult)
                    def split_w(w_ap, tagp):
                        """w -> (hi, lo) f32 APs, hi bf16-valued, w = hi+lo."""
                        h16 = dpool.tile([128, 1], bf16, tag=tagp + "h16")
                        nc.vector.tensor_copy(h16[:], w_ap)
                        h32 = dpool.tile([128, 1], fp32, tag=tagp + "h32")
                        nc.vector.tensor_copy(h32[:], h16[:])
                        lo = dpool.tile([128, 1], fp32, tag=tagp + "lo")
                        nc.vector.tensor_tensor(lo[:], w_ap, h32[:],
                                                Alu.subtract)
                        return h32, lo

                    walo = dpool.tile([128, 1], fp32, tag="walo")
                    nc.vector.tensor_tensor(walo[:], wslo_v[:], zq, Alu.mult)
                    wahi = dpool.tile([128, 1], fp32, tag="wahi")
                    nc.vector.tensor_tensor(wahi[:], wshi_v[:], zq, Alu.mult)
                    wm0 = dpool.tile([128, 1], fp32, tag="wm0")
                    nc.vector.tensor_tensor(wm0[:], vv[:], zq, Alu.mult)
                    walo_h, walo_l = split_w(walo[:], "walo")
                    wahi_h, wahi_l = split_w(wahi[:], "wahi")
                    wm0_h, wm0_l = split_w(wm0[:], "wm0")

                    # ---- U (rows) and V (cols) indicators ----
                    tU = spool.tile([128, HO], fp32, tag="tU")
                    nc.vector.tensor_scalar(tU[:], iota_f[:], xmax[:], None,
                                            Alu.is_lt)
                    U = spool.tile([128, HO], fp32, tag="U")
                    nc.vector.scalar_tensor_tensor(
                        U[:], iota_f[:], xmin[:], tU[:],
                        Alu.is_ge, Alu.logical_and)
                    tV = spool.tile([128, HO], fp32, tag="tV")
                    nc.vector.tensor_scalar(tV[:], iota_f[:], ymax[:], None,
                                            Alu.is_lt)
                    V = spool.tile([128, HO], fp32, tag="V")
                    nc.vector.scalar_tensor_tensor(
                        V[:], iota_f[:], ymin[:], tV[:],
                        Alu.is_ge, Alu.logical_and)

                    U_slo = spool.tile([128, HO], bf16, tag="Uslo")
                    nc.vector.tensor_scalar(U_slo[:], U[:], wslo_v[:], None,
                                            Alu.mult)
                    U_shi = spool.tile([128, HO], bf16, tag="Ushi")
                    nc.vector.tensor_scalar(U_shi[:], U[:], wshi_v[:], None,
                                            Alu.mult)
                    V_bf = spool.tile([128, HO], bf16, tag="Vbf")
                    nc.vector.tensor_copy(V_bf[:], V[:])
                    U_alo_h = spool.tile([128, HO], bf16, tag="Ualoh")
                    nc.vector.tensor_scalar(U_alo_h[:], U[:], walo_h[:], None,
                                            Alu.mult)
                    U_alo_l = spool.tile([128, HO], bf16, tag="Ualol")
                    nc.vector.tensor_scalar(U_alo_l[:], U[:], walo_l[:], None,
                                            Alu.mult)
                    U_ahi_h = spool.tile([128, HO], bf16, tag="Uahih")
                    nc.vector.tensor_scalar(U_ahi_h[:], U[:], wahi_h[:], None,
                                            Alu.mult)
                    U_ahi_l = spool.tile([128, HO], bf16, tag="Uahil")
                    nc.vector.tensor_scalar(U_ahi_l[:], U[:], wahi_l[:], None,
                                            Alu.mult)
                    U_m0_h = spool.tile([128, HO], bf16, tag="Um0h")
                    nc.vector.tensor_scalar(U_m0_h[:], U[:], wm0_h[:], None,
                                            Alu.mult)
                    U_m0_l = spool.tile([128, HO], bf16, tag="Um0l")
                    nc.vector.tensor_scalar(U_m0_l[:], U[:], wm0_l[:], None,
                                            Alu.mult)

                    # ---- per-rowtile paints + decode + loss ----
                    # PSUM tiles are [128, 640]; matmuls write the
                    # bank-aligned slices [0:512] and [512:640].
                    for m in range(5):
                        ms = slice(m * 128, (m + 1) * 128)
                        idx = ((rep * SPC + s) * 5) + m
                        BANKS = (slice(0, 512), slice(512, 640))

                        # wave 1: S-paints (bf16, exact powers of two)
                        T1 = ppool.tile([128, HO], fp32, tag="T1")
                        T2 = ppool.tile([128, HO], fp32, tag="T2")
                        for hs in BANKS:
                            nc.tensor.matmul(T1[:, hs], U_slo[:, ms],
                                             V_bf[:, hs],
                                             start=True, stop=True)
                            nc.tensor.matmul(T2[:, hs], U_shi[:, ms],
                                             V_bf[:, hs],
                                             start=True, stop=True)
                        Cs = dpool.tile([128, HO], fp32, tag="Cs")
                        nc.scalar.mul(Cs[:], T1[:], TAIL)
                        Cs2 = dpool.tile([128, HO], fp32, tag="Cs2")
                        nc.vector.tensor_tensor(Cs2[:], Cs[:], T2[:], Alu.add)

                        # GPSIMD: exponent isolate + exact a = C - E + denom
                        Et = dpool.tile([128, HO], fp32, tag="Et")
                        nc.vector.tensor_scalar(
                            Et[:].bitcast(i32), Cs2[:].bitcast(i32),
                            MASK_EXP, None, Alu.bitwise_and)
                        at = dpool.tile([128, HO], fp32, tag="at")
                        nc.vector.tensor_tensor(at[:], Cs2[:], Et[:],
                                                Alu.subtract)
                        den = dpool.tile([128, HO], fp32, tag="den")
                        nc.vector.scalar_tensor_tensor(
                            den[:], Et[:], 2.0, Cs2[:],
                            Alu.mult, Alu.subtract)
                        nc.vector.tensor_scalar(den[:], den[:], 1e-30,
                                                None, Alu.max)

                        # wave 2: A-paints (split-z bf16 pairs, accumulate)
                        T1b = ppool.tile([128, HO], fp32, tag="T1")
                        T2b = ppool.tile([128, HO], fp32, tag="T2")
                        for hs in BANKS:
                            nc.tensor.matmul(T1b[:, hs], U_alo_h[:, ms],
                                             V_bf[:, hs],
                                             start=True, stop=False)
                            nc.tensor.matmul(T1b[:, hs], U_alo_l[:, ms],
                                             V_bf[:, hs],
                                             start=False, stop=True)
                            nc.tensor.matmul(T2b[:, hs], U_ahi_h[:, ms],
                                             V_bf[:, hs],
                                             start=True, stop=False)
                            nc.tensor.matmul(T2b[:, hs], U_ahi_l[:, ms],
                                             V_bf[:, hs],
                                             start=False, stop=True)
                        CA = dpool.tile([128, HO], fp32, tag="CAt")
                        nc.scalar.mul(CA[:], T1b[:], TAIL)
                        CA2 = dpool.tile([128, HO], fp32, tag="CA2")
                        nc.vector.tensor_tensor(CA2[:], CA[:], T2b[:],
                                                Alu.add)

                        # wave 3: M0 paint + resized feature
                        T1c = ppool.tile([128, HO], fp32, tag="T1")
                        T2c = ppool.tile([128, HO], fp32, tag="T2")
                        for hs in BANKS:
                            nc.tensor.matmul(T1c[:, hs], U_m0_h[:, ms],
                                             V_bf[:, hs],
                                             start=True, stop=False)
                            nc.tensor.matmul(T1c[:, hs], U_m0_l[:, ms],
                                             V_bf[:, hs],
                                             start=False, stop=True)
                            nc.tensor.matmul(T2c[:, hs], out1a[:, ms],
                                             A0[:, hs],
                                             start=True, stop=False)
                            nc.tensor.matmul(T2c[:, hs], out1b[:, ms],
                                             A1[:, hs],
                                             start=False, stop=True)

                        bt = dpool.tile([128, HO], fp32, tag="bt")
                        nc.vector.tensor_tensor(bt[:], at[:], T1c[:],
                                                Alu.mult)
                        numer = dpool.tile([128, HO], fp32, tag="numer")
                        nc.vector.tensor_tensor(numer[:], CA2[:], bt[:],
                                                Alu.subtract)
                        rden = dpool.tile([128, HO], fp32, tag="rden")
                        nc.vector.reciprocal(rden[:], den[:])
                        Z0 = dpool.tile([128, HO], fp32, tag="Z0")
                        nc.vector.tensor_tensor(Z0[:], numer[:], rden[:],
                                                Alu.mult)
                        nc.vector.tensor_scalar(Z0[:], Z0[:], -2.0, 2.0,
                                                Alu.max, Alu.min)
                        dt_ = dpool.tile([128, HO], fp32, tag="dt_")
                        nc.vector.tensor_tensor(dt_[:], T2c[:], Z0[:],
                                                Alu.subtract)
                        # ACT: square + accumulate
                        dsq = dpool.tile([128, HO], fp32, tag="dsq")
                        nc.scalar.activation(
                            dsq[:], dt_[:],
                            mybir.ActivationFunctionType.Square,
                            accum_out=accbuf[:, idx:idx + 1])

            # ---- final reduction ----
            tot = cpool.tile([128, 1], fp32, tag="tot")
            nc.vector.tensor_reduce(
                tot[:], accbuf[:, 0:krep * SPC * 5],
                mybir.AxisListType.X, Alu.add)
            if krep > 1:
                nc.vector.tensor_scalar(tot[:], tot[:], 1.0 / krep, None,
                                        Alu.mult)
            pfin = fpool.tile([128, 320], fp32, tag="paux")
            nc.tensor.matmul(pfin[0:1, 0:1], tot[:], ones_t[:],
                             start=True, stop=True)
            res = cpool.tile([1, 1], fp32, tag="res")
            nc.scalar.copy(res[:], pfin[0:1, 0:1])
            nc.sync.dma_start(out_d.ap(), res[:])

    nc.compile()
    return nc


def _get_nc(krep=1):
    key = ("nc", krep)
    if key not in _CACHE:
        _CACHE[key] = _build(krep)
    return _CACHE[key]


def run_cores(feat, gt_bboxes, krep=1):
    """Run the SPMD kernel; returns list of per-core sum-of-squared-diffs."""
    from concourse.bass_utils import run_bass_kernel_spmd
    nc = _get_nc(krep)
    amat = _resize_matrix()
    feat = np.ascontiguousarray(np.asarray(feat, dtype=np.float32))
    gt = np.ascontiguousarray(np.asarray(gt_bboxes, dtype=np.float32))
    in_maps = []
    for i in range(NCORES):
        sl = slice(i * SPC, (i + 1) * SPC)
        in_maps.append({
            "feat": np.ascontiguousarray(feat[sl, 0]),
            "boxes": np.ascontiguousarray(gt[sl]),
            "amat": amat,
        })
    res = run_bass_kernel_spmd(nc, in_maps, core_ids=list(range(NCORES)))
    return [float(res.results[i]["out"][0, 0]) for i in range(NCORES)]


def kernel(feat, gt_bboxes):
    parts = run_cores(feat, gt_bboxes, krep=1)
    total = float(np.sum(np.asarray(parts, dtype=np.float64)))
    return np.asarray(np.float32(total / NPIX))


# revision 23
# speedup vs baseline: 366.2848x; 1.0692x over previous
# Trainium2 Bass kernel for nn_CFTAuxHead (bilinear 4x resize + bbox
# rasterization + MSE loss), data-parallel over batch across 8 NeuronCores.
#
# Math summary (per sample):
#   feat_up = A^T @ feat @ A  (A = exact 160->640 bilinear weight matrix)
#   heatmap = last-writer-wins paint of 128 axis-aligned rects (value z_n)
#   loss    = mean((feat_up - heatmap)^2) over all pixels
#
# Rasterization on device: 5 "paint" matmuls over box interval-indicator
# matrices U[n, row], V[n, col] with per-box weights:
#   S_lo = sum_n 2^(n')   (n' = n mod 64, boxes n < 64)      [exponent encode]
#   S_hi = sum_n 2^(n')   (boxes n >= 64)
#   A_lo/A_hi = same with z_n * 2^(n')
#   M0   = sum_n z_n
# Per-pixel decode (exact when coverage depth <= 2, clamped otherwise):
#   C  = S_hi + 2^-64 * S_lo        CA = A_hi + 2^-64 * A_lo
#   E  = C & 0xFF800000             (isolates 2^(top box index), exact)
#   Z  = clamp((CA - (C - E) * M0) / (2E - C), -2, 2);  Z = 0 if uncovered
# loss contribution = (feat_up - Z)^2, reduced on-chip to one scalar per core.

import os
import numpy as np

B, C_IN, H, W = 32, 1, 160, 160
UP = 4
HO, WO = H * UP, W * UP
NBOX = 128
NCORES = 8
SPC = B // NCORES  # samples per core
NPIX = float(B * HO * WO)

_CACHE = {}


def _resize_matrix():
    """Exact bilinear (half-pixel centers, edge-clamped) 160->640 matrix,
    matching jax.image.resize(method='bilinear') for upsampling."""
    n_in, n_out = H, HO
    scale = n_out / n_in
    x = (np.arange(n_out, dtype=np.float64) + 0.5) / scale - 0.5
    k = np.arange(n_in, dtype=np.float64)
    w = np.maximum(0.0, 1.0 - np.abs(x[None, :] - k[:, None]))  # [in, out]
    w = w / w.sum(axis=0, keepdims=True)
    return w.astype(np.float32)


def _build(krep=1):
    import concourse.bacc as bacc
    import concourse.mybir as mybir
    from concourse.tile import TileContext

    skip_decode = os.environ.get("KV_SKIP_DECODE", "0") == "1"
    skip_mm = os.environ.get("KV_SKIP_MM", "0") == "1"

    fp32 = mybir.dt.float32
    bf16 = mybir.dt.bfloat16
    i32 = mybir.dt.int32
    Alu = mybir.AluOpType

    nc = bacc.Bacc("TRN2", target_bir_lowering=False, debug=False,
                   enable_asserts=False, num_devices=NCORES)
    feat_d = nc.dram_tensor("feat", [SPC, H, W], fp32, kind="ExternalInput")
    box_d = nc.dram_tensor("boxes", [SPC, NBOX, 5], fp32, kind="ExternalInput")
    amat_d = nc.dram_tensor("amat", [H, HO], fp32, kind="ExternalInput")
    out_d = nc.dram_tensor("out", [1, 1], fp32, kind="ExternalOutput")

    TAIL = float(2.0 ** -64)
    MASK_EXP = -8388608  # 0xFF800000 as signed int32

    with TileContext(nc, num_cores=NCORES) as tc:
        with tc.tile_pool(name="const", bufs=1) as cpool, \
             tc.tile_pool(name="samp", bufs=2) as spool, \
             tc.tile_pool(name="dec", bufs=3) as dpool, \
             tc.tile_pool(name="ps", bufs=1, space="PSUM") as ppool, \
             tc.tile_pool(name="psf", bufs=1, space="PSUM") as fpool:

            # ---- constants ----
            A0 = cpool.tile([128, HO], fp32, tag="A0")
            A1 = cpool.tile([32, HO], fp32, tag="A1")
            nc.sync.dma_start(A0[:], amat_d.ap()[0:128, :])
            nc.sync.dma_start(A1[:], amat_d.ap()[128:160, :])

            iota_i = cpool.tile([128, HO], i32, tag="ioti")
            nc.gpsimd.iota(iota_i[:], pattern=[[1, HO]], base=0,
                           channel_multiplier=0)
            iota_f = cpool.tile([128, HO], fp32, tag="iotf")
            nc.vector.tensor_copy(iota_f[:], iota_i[:])

            nidx_i = cpool.tile([128, 1], i32, tag="nidxi")
            nc.gpsimd.iota(nidx_i[:], pattern=[[1, 1]], base=1,
                           channel_multiplier=1)  # n' = n+1 in 1..128
            nidx_f = cpool.tile([128, 1], fp32, tag="nidxf")
            nc.vector.tensor_copy(nidx_f[:], nidx_i[:])

            ones_t = cpool.tile([128, 1], fp32, tag="ones")
            nc.vector.memset(ones_t[:], 1.0)
            eps_t = cpool.tile([128, 1], fp32, tag="epsb")
            nc.vector.memset(eps_t[:], float(2.0 ** -94))

            # group masks and exponent weights
            glo = cpool.tile([128, 1], fp32, tag="glo")
            nc.vector.tensor_scalar(glo[:], nidx_f[:], 64.0, None, Alu.is_le)
            ghi = cpool.tile([128, 1], fp32, tag="ghi")
            nc.vector.tensor_scalar(ghi[:], nidx_f[:], 64.0, None, Alu.is_gt)

            wslo_b = cpool.tile([128, 1], i32, tag="wslob")
            nc.vector.tensor_scalar(wslo_b[:], nidx_i[:], 126, None, Alu.add)
            nc.vector.tensor_scalar(wslo_b[:], wslo_b[:], 23, None,
                                    Alu.logical_shift_left)
            wshi_b = cpool.tile([128, 1], i32, tag="wshib")
            nc.vector.tensor_scalar(wshi_b[:], nidx_i[:], 62, None, Alu.add)
            nc.vector.tensor_scalar(wshi_b[:], wshi_b[:], 23, None,
                                    Alu.logical_shift_left)
            wslo = cpool.tile([128, 1], fp32, tag="wslo")
            nc.vector.tensor_tensor(wslo[:], wslo_b[:].bitcast(fp32), glo[:],
                                    Alu.mult)
            wshi = cpool.tile([128, 1], fp32, tag="wshi")
            nc.vector.tensor_tensor(wshi[:], wshi_b[:].bitcast(fp32), ghi[:],
                                    Alu.mult)

            accbuf = cpool.tile([128, krep * SPC * 5], fp32, tag="acc")

            def floor_pos(src_ap, tagp):
                """floor(x) for 0 <= x < 2^23, robust to convert rounding."""
                ti = dpool.tile([128, 1], i32, tag=tagp + "_i")
                nc.vector.tensor_copy(ti[:], src_ap)
                tf = dpool.tile([128, 1], fp32, tag=tagp + "_f")
                nc.vector.tensor_copy(tf[:], ti[:])
                m = dpool.tile([128, 1], fp32, tag=tagp + "_m")
                nc.vector.tensor_tensor(m[:], tf[:], src_ap, Alu.is_gt)
                fl = dpool.tile([128, 1], fp32, tag=tagp + "_o")
                nc.vector.tensor_tensor(fl[:], tf[:], m[:], Alu.subtract)
                return fl

            for rep in range(krep):
                for s in range(SPC):
                    # ---- load feat, resize step 1: out1 = F^T A ----
                    F0 = spool.tile([128, W], fp32, tag="F0")
                    F1 = spool.tile([32, W], fp32, tag="F1")
                    nc.sync.dma_start(F0[:], feat_d.ap()[s, 0:128, :])
                    nc.sync.dma_start(F1[:], feat_d.ap()[s, 128:160, :])

                    out1a = spool.tile([128, HO], fp32, tag="out1a")
                    out1b = spool.tile([32, HO], fp32, tag="out1b")
                    for mc, (msz, o1) in enumerate([(128, out1a), (32, out1b)]):
                        moff = 0 if mc == 0 else 128
                        for hh in range(2):
                            hs = slice(hh * 320, (hh + 1) * 320)
                            p1 = fpool.tile([128, 320], fp32, tag="paux")
                            nc.tensor.matmul(
                                p1[0:msz, :], F0[:, moff:moff + msz], A0[:, hs],
                                start=True, stop=False)
                            nc.tensor.matmul(
                                p1[0:msz, :], F1[:, moff:moff + msz], A1[:, hs],
                                start=False, stop=True)
                            nc.scalar.copy(o1[:, hs], p1[0:msz, :])

                    # ---- box prep ----
                    bx = spool.tile([128, 5], fp32, tag="bx")
                    nc.sync.dma_start(bx[:], box_d.ap()[s])
                    xq = bx[:, 0:1]
                    yq = bx[:, 1:2]
                    zq = bx[:, 2:3]
                    wq = bx[:, 3:4]
                    lq = bx[:, 4:5]

                    w2 = dpool.tile([128, 1], fp32, tag="w2")
                    nc.vector.tensor_scalar(w2[:], wq, 0.5, None, Alu.mult)
                    l2 = dpool.tile([128, 1], fp32, tag="l2")
                    nc.vector.tensor_scalar(l2[:], lq, 0.5, None, Alu.mult)

                    cx = floor_pos(xq, "cx")
                    cy = floor_pos(yq, "cy")
                    hw = floor_pos(w2[:], "hw")
                    hl = floor_pos(l2[:], "hl")
                    nc.vector.tensor_scalar(hw[:], hw[:], 3.0, None, Alu.max)
                    nc.vector.tensor_scalar(hl[:], hl[:], 3.0, None, Alu.max)

                    xmin = dpool.tile([128, 1], fp32, tag="xmin")
                    nc.vector.tensor_tensor(xmin[:], cx[:], hw[:], Alu.subtract)
                    nc.vector.tensor_scalar(xmin[:], xmin[:], 0.0, None, Alu.max)
                    xmax = dpool.tile([128, 1], fp32, tag="xmax")
                    nc.vector.tensor_tensor(xmax[:], cx[:], hw[:], Alu.add)
                    nc.vector.tensor_scalar(xmax[:], xmax[:], 1.0, float(HO),
                                            Alu.add, Alu.min)
                    ymin = dpool.tile([128, 1], fp32, tag="ymin")
                    nc.vector.tensor_tensor(ymin[:], cy[:], hl[:], Alu.subtract)
                    nc.vector.tensor_scalar(ymin[:], ymin[:], 0.0, None, Alu.max)
                    ymax = dpool.tile([128, 1], fp32, tag="ymax")
                    nc.vector.tensor_tensor(ymax[:], cy[:], hl[:], Alu.add)
                    nc.vector.tensor_scalar(ymax[:], ymax[:], 1.0, float(WO),
                                            Alu.add, Alu.min)

                    # validity (w > 0 and l > 0) folded into U weights
                    vw = dpool.tile([128, 1], fp32, tag="vw")
                    nc.vector.tensor_scalar(vw[:], wq, 0.0, None, Alu.is_gt)
                    vl = dpool.tile([128, 1], fp32, tag="vl")
                    nc.vector.tensor_scalar(vl[:], lq, 0.0, None, Alu.is_gt)
                    vv = dpool.tile([128, 1], fp32, tag="vv")
                    nc.vector.tensor_tensor(vv[:], vw[:], vl[:], Alu.mult)

                    # per-box paint weights (valid-masked)
                    wslo_v = dpool.tile([128, 1], fp32, tag="wslov")
                    nc.vector.tensor_tensor(wslo_v[:], wslo[:], vv[:], Alu.mult)
                    wshi_v = dpool.tile([128, 1], fp32, tag="wshiv")
                    nc.vector.tensor_tensor(wshi_v[:], wshi[:], vv[:], Alu.mult)
                    def split_w(w_ap, tagp):
                        """w -> (hi, lo) f32 APs, hi bf16-valued, w = hi+lo."""
                        h16 = dpool.tile([128, 1], bf16, tag=tagp + "h16")
                        nc.vector.tensor_copy(h16[:], w_ap)
                        h32 = dpool.tile([128, 1], fp32, tag=tagp + "h32")
                        nc.vector.tensor_copy(h32[:], h16[:])
                        lo = dpool.tile([128, 1], fp32, tag=tagp + "lo")
                        nc.vector.tensor_tensor(lo[:], w_ap, h32[:],
                                                Alu.subtract)
                        return h32, lo

                    walo = dpool.tile([128, 1], fp32, tag="walo")
                    nc.vector.tensor_tensor(walo[:], wslo_v[:], zq, Alu.mult)
                    wahi = dpool.tile([128, 1], fp32, tag="wahi")
                    nc.vector.tensor_tensor(wahi[:], wshi_v[:], zq, Alu.mult)
                    wm0 = dpool.tile([128, 1], fp32, tag="wm0")
                    nc.vector.tensor_tensor(wm0[:], vv[:], zq, Alu.mult)
                    walo_h, walo_l = split_w(walo[:], "walo")
                    wahi_h, wahi_l = split_w(wahi[:], "wahi")
                    wm0_h, wm0_l = split_w(wm0[:], "wm0")

                    # ---- U (rows) and V (cols) indicators ----
                    tU = spool.tile([128, HO], fp32, tag="tU")
                    nc.vector.tensor_scalar(tU[:], iota_f[:], xmax[:], None,
                                            Alu.is_lt)
                    U = spool.tile([128, HO], fp32, tag="U")
                    nc.vector.scalar_tensor_tensor(
                        U[:], iota_f[:], xmin[:], tU[:],
                        Alu.is_ge, Alu.logical_and)
                    tV = spool.tile([128, HO], fp32, tag="tV")
                    nc.vector.tensor_scalar(tV[:], iota_f[:], ymax[:], None,
                                            Alu.is_lt)
                    V = spool.tile([128, HO], fp32, tag="V")
                    nc.vector.scalar_tensor_tensor(
                        V[:], iota_f[:], ymin[:], tV[:],
                        Alu.is_ge, Alu.logical_and)

                    U_slo = spool.tile([128, HO], bf16, tag="Uslo")
                    nc.vector.tensor_scalar(U_slo[:], U[:], wslo_v[:], None,
                                            Alu.mult)
                    U_shi = spool.tile([128, HO], bf16, tag="Ushi")
                    nc.vector.tensor_scalar(U_shi[:], U[:], wshi_v[:], None,
                                            Alu.mult)
                    V_bf = spool.tile([128, HO], bf16, tag="Vbf")
                    nc.vector.tensor_copy(V_bf[:], V[:])
                    U_alo_h = spool.tile([128, HO], bf16, tag="Ualoh")
                    nc.vector.tensor_scalar(U_alo_h[:], U[:], walo_h[:], None,
                                            Alu.mult)
                    U_alo_l = spool.tile([128, HO], bf16, tag="Ualol")
                    nc.vector.tensor_scalar(U_alo_l[:], U[:], walo_l[:], None,
                                            Alu.mult)
                    U_ahi_h = spool.tile([128, HO], bf16, tag="Uahih")
                    nc.vector.tensor_scalar(U_ahi_h[:], U[:], wahi_h[:], None,
                                            Alu.mult)
                    U_ahi_l = spool.tile([128, HO], bf16, tag="Uahil")
                    nc.vector.tensor_scalar(U_ahi_l[:], U[:], wahi_l[:], None,
                                            Alu.mult)
                    U_m0_h = spool.tile([128, HO], bf16, tag="Um0h")
                    nc.vector.tensor_scalar(U_m0_h[:], U[:], wm0_h[:], None,
                                            Alu.mult)
                    U_m0_l = spool.tile([128, HO], bf16, tag="Um0l")
                    nc.vector.tensor_scalar(U_m0_l[:], U[:], wm0_l[:], None,
                                            Alu.mult)

                    # ---- per-rowtile paints + decode + loss ----
                    # PSUM tiles are [128, 640]; matmuls write the
                    # bank-aligned slices [0:512] and [512:640].
                    for m in range(5):
                        ms = slice(m * 128, (m + 1) * 128)
                        idx = ((rep * SPC + s) * 5) + m
                        BANKS = (slice(0, 512), slice(512, 640))

                        # wave 1: S-paints (bf16, exact powers of two)
                        T1 = ppool.tile([128, HO], fp32, tag="T1")
                        T2 = ppool.tile([128, HO], fp32, tag="T2")
                        for hs in BANKS:
                            nc.tensor.matmul(T1[:, hs], U_slo[:, ms],
                                             V_bf[:, hs],
                                             start=True, stop=True)
                            nc.tensor.matmul(T2[:, hs], U_shi[:, ms],
                                             V_bf[:, hs],
                                             start=True, stop=True)
                        # eps floor: uncovered pixels get C = eps (a virtual
                        # empty box far below every real weight), so the
                        # decode yields Z = 0/eps = 0 with no max() guard.
                        Cs = dpool.tile([128, HO], fp32, tag="Cs")
                        nc.scalar.activation(
                            Cs[:], T1[:], mybir.ActivationFunctionType.Identity,
                            bias=eps_t[:], scale=TAIL)
                        Cs2 = dpool.tile([128, HO], fp32, tag="Cs2")
                        nc.vector.tensor_tensor(Cs2[:], Cs[:], T2[:], Alu.add)

                        Et = dpool.tile([128, HO], fp32, tag="Et")
                        nc.vector.tensor_scalar(
                            Et[:].bitcast(i32), Cs2[:].bitcast(i32),
                            MASK_EXP, None, Alu.bitwise_and)
                        at = dpool.tile([128, HO], fp32, tag="at")
                        nc.gpsimd.tensor_tensor(at[:], Cs2[:], Et[:],
                                                Alu.subtract)
                        den = dpool.tile([128, HO], fp32, tag="den")
                        nc.vector.scalar_tensor_tensor(
                            den[:], Et[:], 2.0, Cs2[:],
                            Alu.mult, Alu.subtract)

                        # wave 2: A-paints (split-z bf16 pairs, accumulate)
                        T1b = ppool.tile([128, HO], fp32, tag="T1")
                        T2b = ppool.tile([128, HO], fp32, tag="T2")
                        for hs in BANKS:
                            nc.tensor.matmul(T1b[:, hs], U_alo_h[:, ms],
                                             V_bf[:, hs],
                                             start=True, stop=False)
                            nc.tensor.matmul(T1b[:, hs], U_alo_l[:, ms],
                                             V_bf[:, hs],
                                             start=False, stop=True)
                            nc.tensor.matmul(T2b[:, hs], U_ahi_h[:, ms],
                                             V_bf[:, hs],
                                             start=True, stop=False)
                            nc.tensor.matmul(T2b[:, hs], U_ahi_l[:, ms],
                                             V_bf[:, hs],
                                             start=False, stop=True)
                        CA = dpool.tile([128, HO], fp32, tag="CAt")
                        nc.scalar.mul(CA[:], T1b[:], TAIL)
                        CA2 = dpool.tile([128, HO], fp32, tag="CA2")
                        nc.vector.tensor_tensor(CA2[:], CA[:], T2b[:],
                                                Alu.add)

                        # wave 3: M0 paint + resized feature
                        T1c = ppool.tile([128, HO], fp32, tag="T1")
                        T2c = ppool.tile([128, HO], fp32, tag="T2")
                        for hs in BANKS:
                            nc.tensor.matmul(T1c[:, hs], U_m0_h[:, ms],
                                             V_bf[:, hs],
                                             start=True, stop=False)
                            nc.tensor.matmul(T1c[:, hs], U_m0_l[:, ms],
                                             V_bf[:, hs],
                                             start=False, stop=True)
                            nc.tensor.matmul(T2c[:, hs], out1a[:, ms],
                                             A0[:, hs],
                                             start=True, stop=False)
                            nc.tensor.matmul(T2c[:, hs], out1b[:, ms],
                                             A1[:, hs],
                                             start=False, stop=True)

                        bt = dpool.tile([128, HO], fp32, tag="bt")
                        nc.vector.tensor_tensor(bt[:], at[:], T1c[:],
                                                Alu.mult)
                        numer = dpool.tile([128, HO], fp32, tag="numer")
                        nc.gpsimd.tensor_tensor(numer[:], CA2[:], bt[:],
                                                Alu.subtract)
                        rden = dpool.tile([128, HO], fp32, tag="rden")
                        nc.vector.reciprocal(rden[:], den[:])
                        Z0 = dpool.tile([128, HO], fp32, tag="Z0")
                        nc.gpsimd.tensor_tensor(Z0[:], numer[:], rden[:],
                                                Alu.mult)
                        nc.gpsimd.tensor_scalar(Z0[:], Z0[:], -2.0, 2.0,
                                                Alu.max, Alu.min)
                        dt_ = dpool.tile([128, HO], fp32, tag="dt_")
                        nc.vector.tensor_tensor(dt_[:], T2c[:], Z0[:],
                                                Alu.subtract)
                        # ACT: square + accumulate
                        dsq = dpool.tile([128, HO], fp32, tag="dsq")
                        nc.scalar.activation(
                            dsq[:], dt_[:],
                            mybir.ActivationFunctionType.Square,
                            accum_out=accbuf[:, idx:idx + 1])

            # ---- final reduction ----
            tot = cpool.tile([128, 1], fp32, tag="tot")
            nc.vector.tensor_reduce(
                tot[:], accbuf[:, 0:krep * SPC * 5],
                mybir.AxisListType.X, Alu.add)
            if krep > 1:
                nc.vector.tensor_scalar(tot[:], tot[:], 1.0 / krep, None,
                                        Alu.mult)
            pfin = fpool.tile([128, 320], fp32, tag="paux")
            nc.tensor.matmul(pfin[0:1, 0:1], tot[:], ones_t[:],
                             start=True, stop=True)
            res = cpool.tile([1, 1], fp32, tag="res")
            nc.scalar.copy(res[:], pfin[0:1, 0:1])
            nc.sync.dma_start(out_d.ap(), res[:])

    nc.compile()
    return nc


def _get_nc(krep=1):
    key = ("nc", krep)
    if key not in _CACHE:
        _CACHE[key] = _build(krep)
    return _CACHE[key]


def run_cores(feat, gt_bboxes, krep=1):
    """Run the SPMD kernel; returns list of per-core sum-of-squared-diffs."""
    from concourse.bass_utils import run_bass_kernel_spmd
    nc = _get_nc(krep)
    amat = _resize_matrix()
    feat = np.ascontiguousarray(np.asarray(feat, dtype=np.float32))
    gt = np.ascontiguousarray(np.asarray(gt_bboxes, dtype=np.float32))
    in_maps = []
    for i in range(NCORES):
        sl = slice(i * SPC, (i + 1) * SPC)
        in_maps.append({
            "feat": np.ascontiguousarray(feat[sl, 0]),
            "boxes": np.ascontiguousarray(gt[sl]),
            "amat": amat,
        })
    res = run_bass_kernel_spmd(nc, in_maps, core_ids=list(range(NCORES)))
    return [float(res.results[i]["out"][0, 0]) for i in range(NCORES)]


def kernel(feat, gt_bboxes):
    parts = run_cores(feat, gt_bboxes, krep=1)
    total = float(np.sum(np.asarray(parts, dtype=np.float64)))
    return np.asarray(np.float32(total / NPIX))
